# revision 24
# baseline (speedup 1.0000x reference)
"""Self-contained Trainium2 Bass kernel for causal multi-head attention.

Problem: B=2, S=2048, D=1024, H=16 heads (dk=64), fp32, causal + padding mask.
Sharding across 8 NeuronCores: core c -> batch c//4, head-group c%4 (4 heads).

Device-side design (v3):
  - All matmul operands bf16 (inputs cast on host) except the softmax
    normalization chain; fp32 PSUM accumulation throughout.
  - qT/kT transposed [dk, S]; scores S_T[k, q] with the head pair
    row-packed into array rows 0-63 / 64-127 (concurrent bf16 streams).
  - Padding via the exp's per-partition bias (-1e9 on padded keys), so V
    needs no masking and the denominator (ones column 64 of the PV
    stationary operand) is correct.
  - Causal: additive -8e9 triangle on diagonal 128-blocks (pre-scale).
  - Normalization with no cross-engine round-trips: ctx^T+den land in
    PSUM rows 0-64, copied once to SBUF (fp32r); reciprocal in place on
    den row 64; a K=1 matmul from partition 64 broadcasts 1/den to
    partitions 0-63 (tile_position (64,0)); DVE multiplies into the
    bf16 ctx pair tile. Head 1's half is DMA-shifted to partitions
    64-127 so the out-projection packs the pair into one K=128 chain.
  - Input DMAs: tiny tensors first, bulk bf16 split across both HWDGE
    queues (sync + scalar); attention for query-chunk 0 is emitted
    between projection chunks so the PE stays busy (and the HAM clock
    gate warm) while the input streams.
Fully-masked rows (all keys up to q padded) produce garbage on device
and are overwritten on host with the uniform-attention reference value.
"""

import numpy as np
from contextlib import ExitStack

import concourse.bass as bass
import concourse.bacc as bacc
import concourse.tile as tile
import concourse.mybir as mybir
from concourse.bass import ds, ts

F32 = mybir.dt.float32
FR = mybir.dt.float32r
BF = mybir.dt.bfloat16
AF = mybir.ActivationFunctionType

P = 128
S = 2048
D = 1024
HL = 4          # heads per core
DK = 64
KT = D // P     # 8 k-tiles over the model dim
ST = S // P     # 16 seq tiles
NQC = 4         # 512-wide query chunks
NEG = -8.0e9    # pre-scale causal mask value; *0.125 = -1e9 -> exp = 0
PADNEG = -1.0e9  # post-scale padding bias; exp(-1e9) = 0
N_CORES = 8
N_HEAD = 16


def build_program(num_devices=N_CORES):
    nc = bacc.Bacc(
        "TRN2",
        target_bir_lowering=False,
        debug=False,
        enable_asserts=True,
        num_devices=num_devices,
    )
    ins = {
        "xt": nc.dram_tensor("xt", [D, S], BF, kind="ExternalInput").ap(),
        "wq": nc.dram_tensor("wq", [D, 2 * P], BF, kind="ExternalInput").ap(),
        "wk": nc.dram_tensor("wk", [D, 2 * P], BF, kind="ExternalInput").ap(),
        "wv": nc.dram_tensor("wv", [D, 2 * P], BF, kind="ExternalInput").ap(),
        "wo": nc.dram_tensor("wo", [2 * P, D], BF, kind="ExternalInput").ap(),
        "bq": nc.dram_tensor("bq", [1, 2 * P], BF, kind="ExternalInput").ap(),
        "padb": nc.dram_tensor("padb", [P, ST], F32, kind="ExternalInput").ap(),
        "tri": nc.dram_tensor("tri", [P, P], BF, kind="ExternalInput").ap(),
        "ident": nc.dram_tensor("ident", [P, P], BF, kind="ExternalInput").ap(),
    }
    y = nc.dram_tensor("y", [S, D], F32, kind="ExternalOutput").ap()
    ins["rcp_dram"] = nc.dram_tensor("rcp_dram", [2 * NQC * 2, 512], F32).ap()

    with tile.TileContext(nc) as tc:
        _body(tc, y, ins)

    nc.compile()
    return nc


def _body(tc, y, ins):
    nc = tc.nc

    with ExitStack() as ctx:
        const = ctx.enter_context(tc.tile_pool(name="const", bufs=1))
        pt_pool = ctx.enter_context(tc.tile_pool(name="pt", bufs=3))
        rrp = ctx.enter_context(tc.tile_pool(name="rr", bufs=2))
        ysb = ctx.enter_context(tc.tile_pool(name="ysb", bufs=2))
        psA = ctx.enter_context(tc.tile_pool(name="psA", bufs=2, space="PSUM"))
        psB = ctx.enter_context(tc.tile_pool(name="psB", bufs=2, space="PSUM"))
        psY = ctx.enter_context(tc.tile_pool(name="psY", bufs=2, space="PSUM"))

        # warmup operand memset FIRST so the PE can start immediately
        ones_sb = const.tile([1, 512], FR)
        nc.vector.memset(ones_sb[:].bitcast(F32), 1.0)
        ones_f32 = const.tile([1, DK], F32)
        nc.vector.memset(ones_f32[:], 1.0)
        ones_bf = const.tile([1, 512], BF)
        nc.vector.memset(ones_bf[:], 1.0)

        # ---------------- input DMAs ----------------
        # tiny tensors first so nothing downstream stalls behind the bulk
        bq_sb = const.tile([1, 2 * P], BF)
        nc.sync.dma_start(bq_sb[:], ins["bq"])
        padb_sb = const.tile([P, ST], F32)
        nc.sync.dma_start(padb_sb[:], ins["padb"])
        # causal mask as a matmul: ps += trit.T @ I (trit is the lhsT form)
        trit_sb = const.tile([P, P], BF)
        nc.sync.dma_start(trit_sb[:], ins["tri"])
        ident_sb = const.tile([P, P], BF)
        nc.sync.dma_start(ident_sb[:], ins["ident"])

        xt_sb = const.tile([P, KT, S], BF)
        wq_sb = const.tile([P, KT, 2 * P], BF)
        wk_sb = const.tile([P, KT, 2 * P], BF)
        wv_sb = const.tile([P, KT, 2 * P], BF)
        xt_r = ins["xt"].rearrange("(k p) s -> k p s", p=P)
        w_rs = {n: ins[n].rearrange("(k p) n -> k p n", p=P) for n in ("wq", "wk", "wv")}
        # chunk-0 critical set split across the two HWDGE queues
        for k in range(KT):
            nc.sync.dma_start(wq_sb[:, k], w_rs["wq"][k])
            nc.scalar.dma_start(wk_sb[:, k], w_rs["wk"][k])
            nc.sync.dma_start(wv_sb[:, k], w_rs["wv"][k])
            nc.scalar.dma_start(xt_sb[:, k, 0:512], xt_r[k][:, 0:512])
        for k in range(KT):
            eng = nc.sync if k % 2 == 0 else nc.scalar
            eng.dma_start(xt_sb[:, k, 512:1024], xt_r[k][:, 512:1024])

        # wo: bf16, head pair m stacked as 128 contraction rows
        wo_sb = const.tile([P, 2, D], BF)
        wo_r = ins["wo"].rearrange("(m p) n -> m p n", p=P)
        for m in range(2):
            nc.scalar.dma_start(wo_sb[:, m], wo_r[m])

        for k in range(KT):
            eng = nc.sync if k % 2 == 0 else nc.scalar
            eng.dma_start(xt_sb[:, k, 1024:2048], xt_r[k][:, 1024:2048])

        qt_sb = const.tile([P, 2, S], BF)
        kt_sb = const.tile([P, 2, S], BF)
        # per head: 64 value cols + ones denominator col; padded so a
        # 128-wide stationary slice starting at h*65 stays in bounds (the
        # extra columns produce junk output rows 65-127, never read)
        VW = HL * (DK + 1) + DK - 1  # 323
        vaug_sb = const.tile([P, ST, VW], BF)
        den_cols = vaug_sb[:, :, 0 : HL * (DK + 1)].rearrange(
            "p s (h c) -> p s h c", c=DK + 1
        )[:, :, :, DK : DK + 1]
        nc.vector.memset(den_cols, 1.0)
        nc.vector.memset(vaug_sb[:, :, HL * (DK + 1) : VW], 0.0)

        # normalized context head pairs [128, 512] bf16: rows 0-63 head 2m,
        # rows 64-127 head 2m+1 (DMA-shifted in)
        ctx_sets = []
        for st in range(2):
            tiles = []
            for m in range(2):
                t = const.tile([P, 512], BF, name=f"ctxsb{st}_{m}", tag=f"ctxsb{st}_{m}")
                tiles.append(t)
            ctx_sets.append(tiles)

        # PE warmup while the input DMAs stream (HAM un-throttle needs
        # ~3.4us of sustained matmul activity; these are dep-free)
        def warm(n):
            warm_ps = psY.tile([P, 512], F32, name="warm", tag="yp")
            for i in range(n):
                nc.tensor.matmul(
                    warm_ps[:], ones_sb[:, 0:P], ones_sb[:], start=True, stop=True
                )

        # ---------------- projections for one 512-token chunk ----------------
        def proj_chunk(n):
            for tgt, w_sb, bias in ((qt_sb, wq_sb, True), (kt_sb, wk_sb, False)):
                ps = psA.tile([P, 1024], F32, name=f"ps_p{n}", tag="ps")
                for m in range(2):
                    for k in range(KT):
                        nc.tensor.matmul(
                            ps[:, ts(m, 512)],
                            w_sb[:, k, ts(m, P)],
                            xt_sb[:, k, ds(n * 512, 512)],
                            start=(k == 0),
                            stop=(k == KT - 1) and not bias,
                        )
                    if bias:
                        # + bq as a K=1 rank-1 accumulation
                        nc.tensor.matmul(
                            ps[:, ts(m, 512)],
                            bq_sb[0:1, ds(m * P, P)],
                            ones_bf[:],
                            start=False,
                            stop=True,
                        )
                for m in range(2):
                    nc.vector.tensor_copy(
                        tgt[:, m, ds(n * 512, 512)], ps[:, ts(m, 512)]
                    )
            # V projection rides the (idle during proj) psY pool so the psA
            # ring never couples the V matmuls to the q/k copy tails
            for half in range(2):
                ps = psY.tile([P, 512], F32, name=f"ps_v{n}_{half}", tag="yp")
                for si2 in range(2):
                    s = n * 4 + half * 2 + si2
                    for k in range(KT):
                        nc.tensor.matmul(
                            ps[:, ts(si2, 256)],
                            xt_sb[:, k, ts(s, P)],
                            wv_sb[:, k, :],
                            start=(k == 0),
                            stop=(k == KT - 1),
                        )
                for si2 in range(2):
                    s = n * 4 + half * 2 + si2
                    dst = vaug_sb[:, s, 0 : HL * (DK + 1)].rearrange(
                        "p (h c) -> p h c", c=DK + 1
                    )[:, :, 0:DK]
                    src = ps[:, ds(si2 * 256, 256)].rearrange(
                        "p (h c) -> p h c", c=DK
                    )
                    nc.vector.tensor_copy(dst, src)

        # ---------------- attention for one 512-query chunk ----------------
        y_r = y.rearrange("(t p) n -> t p n", p=P)

        def scores_pair(qc, m):
            """QK^T (row-packed bf16 pair), exp(+pad bias), PV; copies ctx^T
            (+denominator in row 64) to SBUF so the PSUM banks free fast."""
            nkb = 4 * qc + 4
            pvs = [
                psB.tile([P, 512], F32, name=f"ctx{qc}_{m}_{i}", tag="ctx")
                for i in range(2)
            ]
            for kb in range(nkb):
                dd = kb - 4 * qc
                qoff = max(0, dd) * P
                w = 512 - qoff
                ps = psA.tile([P, 1024], F32, name=f"ps_a{qc}_{m}_{kb}", tag="ps")
                for hh in range(2):
                    r0 = hh * DK
                    nc.tensor.matmul(
                        ps[:, hh * 512 + qoff : (hh + 1) * 512],
                        kt_sb[r0 : r0 + DK, m, ds(kb * P, P)],
                        qt_sb[r0 : r0 + DK, m, ds(qc * 512 + qoff, w)],
                        start=True,
                        stop=(dd < 0),
                        skip_group_check=(dd >= 0),
                    )
                if dd >= 0:
                    for hh in range(2):
                        diag = ps[:, hh * 512 + qoff : hh * 512 + qoff + P]
                        nc.tensor.matmul(
                            diag,
                            trit_sb[:],
                            ident_sb[:],
                            start=False,
                            stop=True,
                            skip_group_check=True,
                        )
                pt = pt_pool.tile([P, 1024], BF, name=f"pt{qc}_{m}_{kb}", tag="pt")
                ps3 = ps[:].rearrange("p (h q) -> p h q", h=2)[:, :, qoff:]
                pt3 = pt[:].rearrange("p (h q) -> p h q", h=2)[:, :, qoff:]
                nc.scalar.activation(
                    pt3, ps3, AF.Exp, bias=padb_sb[:, kb : kb + 1], scale=0.125
                )
                for hh in range(2):
                    h = 2 * m + hh
                    nc.tensor.matmul(
                        pvs[hh][:, qoff:],
                        vaug_sb[:, kb, ds(h * (DK + 1), P)],
                        pt[:, hh * 512 + qoff : (hh + 1) * 512],
                        start=(kb == 0),
                        stop=(kb == nkb - 1),
                    )
            craws = []
            for hh in range(2):
                h = 2 * m + hh
                craw = rrp.tile(
                    [DK + 1, 512], BF, name=f"craw{qc}_{h}", tag="craw", bufs=5
                )
                nc.vector.tensor_copy(craw[:], pvs[hh][0 : DK + 1, :])
                craws.append(craw)
            return craws

        def start_norm(qc, m, craws):
            """Kick off the reciprocal-broadcast chain (gpsimd DMAs + one DVE
            reciprocal); the multiplies are deferred to finish_norm one
            half-chunk later so no engine FIFO ever blocks on this chain."""
            den2 = rrp.tile([2, 512], F32, name=f"den2_{qc}_{m}", tag="den2", bufs=2)
            for hh in range(2):
                nc.gpsimd.dma_start(den2[hh : hh + 1, :], craws[hh][DK : DK + 1, :])
            rcp2 = rrp.tile([2, 512], F32, name=f"rcp2_{qc}_{m}", tag="rcp2", bufs=2)
            nc.vector.reciprocal_approx_fast(rcp2[:], den2[:])
            base = (qc * 2 + m) * 2
            nc.gpsimd.dma_start(ins["rcp_dram"][base : base + 2, :], rcp2[:])
            rbs = []
            for hh in range(2):
                rb = rrp.tile(
                    [DK, 512], BF, name=f"rb{qc}_{m}{hh}", tag="rb", bufs=4
                )
                nc.gpsimd.dma_start(
                    rb[:],
                    ins["rcp_dram"][base + hh : base + hh + 1, :].to_broadcast(
                        [DK, 512]
                    ),
                )
                rbs.append(rb)
            return (qc, m, craws, rbs)

        def finish_norm(st):
            qc, m, craws, rbs = st
            ctx_pair = ctx_sets[qc % 2][m]
            tmp1 = rrp.tile([DK, 512], BF, name=f"tmp1_{qc}_{m}", tag="tmp1", bufs=2)
            nc.vector.tensor_mul(tmp1[:], craws[1][0:DK, :], rbs[1][:])
            nc.gpsimd.dma_start(ctx_pair[DK:P, :], tmp1[:])
            nc.vector.tensor_mul(ctx_pair[0:DK, :], craws[0][0:DK, :], rbs[0][:])

        def norm_fast(qc, m, craws):
            """Tail path for the final pair: the PE is idle here, and a K=1
            matmul broadcast has much lower latency than the DRAM-bounce
            DMA chain."""
            ctx_pair = ctx_sets[qc % 2][m]
            tmp1 = rrp.tile([DK, 512], BF, name=f"tmp1_{qc}_{m}", tag="tmp1", bufs=2)
            den2 = rrp.tile([2, 512], F32, name=f"den2_{qc}_{m}", tag="den2", bufs=2)
            for hh in range(2):
                nc.gpsimd.dma_start(den2[hh : hh + 1, :], craws[hh][DK : DK + 1, :])
            rcp2 = rrp.tile([2, 512], F32, name=f"rcp2_{qc}_{m}", tag="rcp2", bufs=2)
            nc.vector.reciprocal_approx_fast(rcp2[:], den2[:])
            rcp_b = rrp.tile([1, 512], F32, name=f"rcpb{qc}_{m}", tag="rcpb", bufs=2)
            nc.gpsimd.dma_start(rcp_b[:], rcp2[1:2, :])
            rb_list = []
            for hh in range(2):
                # bf16 rhs keeps the broadcast matmul single-pass
                rcpbb = rrp.tile(
                    [1, 512], BF, name=f"rcpbb{qc}_{m}{hh}", tag="rcpbb", bufs=2
                )
                nc.vector.tensor_copy(
                    rcpbb[:], rcp2[0:1, :] if hh == 0 else rcp_b[:]
                )
                rb_ps = psB.tile([DK, 512], F32, name=f"rbp{qc}_{m}{hh}", tag="ctx")
                nc.tensor.matmul(
                    rb_ps[:], ones_bf[0:1, 0:DK], rcpbb[:], start=True, stop=True
                )
                rb_list.append(rb_ps)
            # head 1 first so the shift DMA overlaps head 0's multiply
            nc.vector.tensor_mul(tmp1[:], craws[1][0:DK, :], rb_list[1][:])
            nc.gpsimd.dma_start(ctx_pair[DK:P, :], tmp1[:])
            nc.vector.tensor_mul(ctx_pair[0:DK, :], craws[0][0:DK, :], rb_list[0][:])

        def outproj(qc, sis=(0, 1, 2, 3), tail=False):
            for si in sis:
                s = qc * 4 + si
                yt = ysb.tile([P, 1024], F32, name=f"yt{s}", tag="yt")
                for nch in range(2):
                    yp = psY.tile([P, 512], F32, name=f"yp{s}_{nch}", tag="yp")
                    for m in range(2):
                        nc.tensor.matmul(
                            yp[:],
                            ctx_sets[qc % 2][m][:, ts(si, P)],
                            wo_sb[:, m, ds(nch * 512, 512)],
                            start=(m == 0),
                            stop=(m == 1),
                        )
                    # tail: copies split across DVE+ACT, y DMAs across both
                    # queues, so the last tokens drain through parallel paths
                    if tail and nch == 1:
                        nc.scalar.activation(yt[:, ts(nch, 512)], yp[:], AF.Copy)
                        nc.gpsimd.dma_start(
                            y_r[s][:, ds(nch * 512, 512)], yt[:, ts(nch, 512)]
                        )
                    else:
                        nc.vector.tensor_copy(yt[:, ts(nch, 512)], yp[:])
                        nc.sync.dma_start(
                            y_r[s][:, ds(nch * 512, 512)], yt[:, ts(nch, 512)]
                        )

        # ---------------- interleaved schedule ----------------
        # Emission order IS the per-engine execution order. qc0's attention
        # is emitted between the projection chunks so the PE has ready work
        # while the xt DMAs stream; late projection chunks ride inside qc1;
        # each pair's normalize multiplies are deferred one half-chunk
        # (finish_norm) so their reciprocal-broadcast latency is fully
        # hidden, and deferred output projections are emitted before
        # start_norm so the PE FIFO never waits on the chain.
        warm(12)
        proj_chunk(0)
        warm(4)
        pend = start_norm(0, 0, scores_pair(0, 0))
        proj_chunk(1)
        craws = scores_pair(0, 1)
        finish_norm(pend)
        pend = start_norm(0, 1, craws)
        for qc in range(1, NQC):
            for m in range(2):
                craws = scores_pair(qc, m)
                finish_norm(pend)
                outproj(qc - 1, sis=(0, 1) if m == 0 else (2, 3))
                if qc == NQC - 1 and m == 1:
                    # dep-free warm matmuls bridge the final normalize chain
                    # so the HAM clock gate stays at 2.4 GHz into the tail
                    warm(14)
                    norm_fast(qc, m, craws)
                    warm(8)
                else:
                    pend = start_norm(qc, m, craws)
                if qc == 1:
                    proj_chunk(2 + m)
        outproj(NQC - 1, tail=True)


# ---------------- host side ----------------

def make_in_maps(x, padding_mask, Wq, bq, Wk, Wv, Wo):
    """Build the 8 per-core input dicts from full inputs."""
    from ml_dtypes import bfloat16 as np_bf16

    x = np.asarray(x, dtype=np.float32)
    pad = np.asarray(padding_mask)
    # lhsT of the causal-mask matmul: (trit.T)[k, q] = NEG where k > q
    trit = np.where(
        np.arange(P)[None, :] > np.arange(P)[:, None], np.float32(NEG), np.float32(0)
    ).astype(np_bf16)
    ident = np.eye(P, dtype=np.float32).astype(np_bf16)
    in_maps = []
    for c in range(N_CORES):
        b, g = divmod(c, 4)
        R = slice(g * 256, g * 256 + 256)
        padb = np.where(pad[b] != 0, np.float32(0), np.float32(PADNEG)).astype(
            np.float32
        ).reshape(ST, P).T.copy()
        in_maps.append(
            {
                "xt": np.ascontiguousarray(x[b].T).astype(np_bf16),
                "wq": np.ascontiguousarray(
                    np.asarray(Wq, np.float32)[R, :].T
                ).astype(np_bf16),
                "wk": np.ascontiguousarray(
                    np.asarray(Wk, np.float32)[R, :].T
                ).astype(np_bf16),
                "wv": np.ascontiguousarray(
                    np.asarray(Wv, np.float32)[R, :].T
                ).astype(np_bf16),
                "wo": np.ascontiguousarray(
                    np.asarray(Wo, np.float32)[:, R].T
                ).astype(np_bf16),
                "bq": np.asarray(bq, np.float32)[R].reshape(1, 2 * P).astype(
                    np_bf16
                ),
                "padb": padb,
                "tri": trit,
                "ident": ident,
            }
        )
    return in_maps


def postprocess(partials, x, padding_mask, Wv, bv, Wo, bo):
    """Sum per-core partials, add folded bias, fix fully-masked rows."""
    x = np.asarray(x, np.float32)
    pad = np.asarray(padding_mask)
    Wv = np.asarray(Wv, np.float32)
    bv = np.asarray(bv, np.float32)
    Wo = np.asarray(Wo, np.float32)
    bo = np.asarray(bo, np.float32)
    B = x.shape[0]
    y = np.zeros((B, S, D), dtype=np.float32)
    for c in range(N_CORES):
        y[c // 4] += partials[c]
    y += (Wo @ bv + bo)[None, None, :]
    # fully-masked rows (reference: uniform attention over all keys)
    for b in range(B):
        nz = np.flatnonzero(pad[b] != 0)
        q0 = int(nz[0]) if len(nz) else S
        if q0 > 0:
            ctx_u = x[b].mean(axis=0) @ Wv.T + bv
            y[b, :q0, :] = ctx_u @ Wo.T + bo
    return y


_NC_CACHE = {}


def _get_program():
    if "nc" not in _NC_CACHE:
        _NC_CACHE["nc"] = build_program()
    return _NC_CACHE["nc"]


def kernel(
    x, padding_mask, Wq, bq, Wk, bk, Wv, bv, Wo, bo
):
    from concourse.bass_utils import run_bass_kernel_spmd

    nc = _get_program()
    in_maps = make_in_maps(x, padding_mask, Wq, bq, Wk, Wv, Wo)
    res = run_bass_kernel_spmd(nc, in_maps, core_ids=list(range(N_CORES)))
    partials = [res.results[c]["y"] for c in range(N_CORES)]
    return postprocess(partials, x, padding_mask, Wv, bv, Wo, bo)
